# revision 42
# baseline (speedup 1.0000x reference)
"""MoE (B=2,S=2048,D=2048,E=8,K=2,F=4096) on 8 Trainium2 NeuronCores.

Strategy:
  Launch 1 (router, data-parallel over tokens): each core LayerNorms its 512
  tokens and computes router logits with bf16 hi/lo 3-pass matmuls (fp32-grade
  accuracy at bf16 PE rate). Softmax / top-2 / gating weights / aux loss are
  tiny O(tokens*E) ops done on host from the logits.
  Host (the "all-to-all"): tokens are grouped by expert, split into slots, and
  bin-packed onto cores to balance the highly non-uniform expert loads.
  Launch 2 (experts, expert-parallel): each core runs the D->F->D FFN (bf16)
  for its assigned (expert, token-chunk) slots and scales by gating weight.
  Host: scatter-add of the two expert contributions per token.
"""
import numpy as np
import ml_dtypes
import concourse.bacc as bacc
import concourse.tile as tile
from concourse import mybir
from concourse.bass_utils import run_bass_kernel_spmd
import concourse.bass as bass

F32 = mybir.dt.float32
BF16 = mybir.dt.bfloat16

B, S, D = 2, 2048, 2048
E, TOPK, F = 8, 2, 4096
EPS_LN = 1e-5
EPS_AUX = 1e-9
NCORES = 8
NTOK = B * S            # 4096
TPC = NTOK // NCORES    # 512 tokens per core in launch 1
DC = D // 128           # 16 chunks of the model dim
FC = F // 128           # 32 chunks of the FFN dim

def _slot_candidates():
    """Slot-size configs (per core, same on all cores), ordered by total
    capacity then by slot count (fewer slots => less weight DMA)."""
    def up16(v):
        return -(-v // 16) * 16

    cands = []
    for total in range(1024, 4096 + 512, 16):
        if total % 64 == 0:
            cands.append((total, 1, (total,)))
        for a in range(up16((total + 1) // 2), total - 192 + 1, 16):
            b = total - a
            if b >= 192:
                cands.append((total, 2, (a, b)))
        if total <= 1600 and total % 32 == 0:
            for a in range(up16((total + 2) // 3), total - 2 * 192 + 1, 32):
                for b in range(up16((total - a + 1) // 2), min(a, total - a - 192) + 1, 32):
                    c = total - a - b
                    if c >= 192:
                        cands.append((total, 3, (a, b, c)))
    cands.sort(key=lambda t: (t[0], t[1]))
    return [c[2] for c in cands]


def _bf16(a):
    return np.asarray(a, np.float32).astype(ml_dtypes.bfloat16)


def _hilo(a):
    hi = _bf16(a)
    lo = (np.asarray(a, np.float32) - hi.astype(np.float32)).astype(ml_dtypes.bfloat16)
    return hi, lo


def _rne_f32r(v):
    """Round fp32 array to fp32r (round-to-nearest-even, 11-bit mantissa)."""
    u = np.ascontiguousarray(v, dtype=np.float32).view(np.uint32).astype(np.uint64)
    shift = np.uint64(12)
    one = np.uint64(1)
    bias = (one << np.uint64(11)) - one + ((u >> shift) & one)
    u2 = ((u + bias) >> shift) << shift
    return u2.astype(np.uint32).view(np.float32)


def _token_tiles(c):
    """Split a slot of c tokens into near-equal PE-friendly tiles (<=512)."""
    nt = -(-c // 512)
    base = c // nt
    base -= base % 16
    sizes = [base] * nt
    sizes[-1] = c - base * (nt - 1)
    tiles = []
    off = 0
    for s in sizes:
        tiles.append((off, s))
        off += s
    return tiles


F32R = mybir.dt.float32r


def build_l1(debug=False, work_bufs=1, small_bufs=2, ps_bufs=2):
    """Router kernel: x_shard [TPC, D] -> logits^T [E, TPC] per core.

    ln_gamma is folded into rW1 and ln_beta into rb1 on the host, so the
    device LayerNorm is a plain z-score. Matmuls run in fp32r (full PE rate,
    ~11-bit mantissa); near-tie tokens are re-routed exactly on the host.
    """
    nc = bacc.Bacc()
    xs = nc.dram_tensor("xs", [TPC, D], F32, kind="ExternalInput").ap()
    # router weights (fp32 values pre-rounded to fp32r on host), prearranged:
    # rw1 [m-chunk 16][p 128][k-chunk 16][col 128]; rw2 [p 128][k-chunk 16][e 8]
    rw1 = nc.dram_tensor("rw1", [DC, 128, DC, 128], F32R, kind="ExternalInput").ap()
    # rW2 zero-padded from E=8 to 128 output columns (fp32r needs M=128)
    rw2 = nc.dram_tensor("rw2", [128, DC, 128], F32R, kind="ExternalInput").ap()
    rb1 = nc.dram_tensor("rb1", [128, DC], F32, kind="ExternalInput").ap()
    rb2 = nc.dram_tensor("rb2", [E, 1], F32, kind="ExternalInput").ap()
    ident = nc.dram_tensor("ident", [128, 128], F32, kind="ExternalInput").ap()
    logits = nc.dram_tensor("logits", [E, TPC], F32, kind="ExternalOutput").ap()
    if debug:
        xnT_o = nc.dram_tensor("xnT_o", [128, DC, TPC], F32, kind="ExternalOutput").ap()
        hr_o = nc.dram_tensor("hr_o", [128, DC, TPC], F32, kind="ExternalOutput").ap()

    TB = TPC // 128  # token blocks of 128

    with tile.TileContext(nc) as tc:
        with (
            tc.tile_pool(name="main", bufs=1) as main,
            tc.tile_pool(name="work", bufs=work_bufs) as work,
            tc.tile_pool(name="small", bufs=small_bufs) as small,
            tc.tile_pool(name="wpool", bufs=2) as wpool,
            tc.tile_pool(name="ps", bufs=ps_bufs, space="PSUM") as ps,
        ):
            # transposed normalized activations (fp32r): [p=d%128, dc, token]
            xnT = main.tile([128, DC, TPC], F32R, tag="xnT")
            id_t = main.tile([128, 128], F32, tag="id_t")
            nc.sync.dma_start(id_t, ident)

            xs_r = xs.rearrange("(tb p) d -> p tb d", p=128)
            for tb in range(TB):
                xtb = small.tile([128, D], F32, tag="xtb")
                nc.sync.dma_start(xtb, xs_r[:, tb, :])
                ssum = small.tile([128, 1], F32, tag="ssum")
                nc.vector.tensor_reduce(ssum, xtb, axis=mybir.AxisListType.X,
                                        op=mybir.AluOpType.add)
                negmu = small.tile([128, 1], F32, tag="negmu")
                nc.vector.tensor_scalar(negmu, ssum, -1.0 / D, None,
                                        op0=mybir.AluOpType.mult)
                xc = work.tile([128, D], F32, tag="xc")
                nc.vector.tensor_scalar(xc, xtb, negmu, None,
                                        op0=mybir.AluOpType.add)
                # var*D via ACT: square with accumulate (frees the DVE)
                sq = work.tile([128, D], F32, tag="sq")
                vsum = small.tile([128, 1], F32, tag="vsum")
                nc.scalar.activation(sq, xc, mybir.ActivationFunctionType.Square,
                                     accum_out=vsum)
                # std = sqrt(vsum/D + eps)
                varep = small.tile([128, 1], F32, tag="varep")
                nc.vector.tensor_scalar(varep, vsum, 1.0 / D, EPS_LN,
                                        op0=mybir.AluOpType.mult,
                                        op1=mybir.AluOpType.add)
                std = small.tile([128, 1], F32, tag="std")
                nc.scalar.sqrt(std, varep)
                rstd = small.tile([128, 1], F32, tag="rstd")
                nc.vector.reciprocal(rstd, std)
                # xn = xc * rstd  (gamma folded into rW1, beta into rb1)
                xn = work.tile([128, D], F32, tag="xn")
                nc.vector.tensor_scalar(xn, xc, rstd, None,
                                        op0=mybir.AluOpType.mult)
                # PE-transpose each [128,128] block; DVE evac rounds to fp32r
                for dc in range(DC):
                    pt = ps.tile([128, 128], F32, tag="pt")
                    nc.tensor.transpose(pt, xn[:, dc * 128:(dc + 1) * 128], id_t)
                    nc.vector.tensor_copy(xnT[:, dc, tb * 128:(tb + 1) * 128], pt)

            # --- mm1: h^T[m] = relu(sum_k rW1[k,m]^T xn^T[k]), fp32r ---
            rb1_t = main.tile([128, DC], F32, tag="rb1")
            nc.sync.dma_start(rb1_t, rb1)
            hr = main.tile([128, DC, TPC], F32R, tag="hr")
            for m in range(DC):
                w1_t = wpool.tile([128, DC, 128], F32R, tag="w1")
                nc.sync.dma_start(w1_t, rw1[m])
                ph = ps.tile([128, TPC], F32, tag="ph")
                for k in range(DC):
                    nc.tensor.matmul(ph, w1_t[:, k, :], xnT[:, k, :],
                                     start=(k == 0), stop=(k == DC - 1))
                # h = max(ph + rb1[m], 0) rounded to fp32r, one DVE op
                nc.vector.tensor_scalar(hr[:, m, :], ph, rb1_t[:, m : m + 1], 0.0,
                                        op0=mybir.AluOpType.add,
                                        op1=mybir.AluOpType.max)

            # --- mm2 fp32r with rW2 zero-padded to M=128 ---
            rw2_t = main.tile([128, DC, 128], F32R, tag="rw2")
            rb2_t = main.tile([E, 1], F32, tag="rb2")
            nc.sync.dma_start(rw2_t, rw2)
            nc.sync.dma_start(rb2_t, rb2)
            pl = ps.tile([128, TPC], F32, tag="pl")
            for k in range(DC):
                nc.tensor.matmul(pl, rw2_t[:, k, :], hr[:, k, :],
                                 start=(k == 0), stop=(k == DC - 1))
            lsb = small.tile([E, TPC], F32, tag="lsb")
            nc.vector.tensor_scalar(lsb, pl[0:E, :], rb2_t[:, 0:1], None,
                                    op0=mybir.AluOpType.add)
            nc.sync.dma_start(logits, lsb)
            if debug:
                nc.sync.dma_start(xnT_o, xnT.bitcast(F32))
                nc.sync.dma_start(hr_o, hr.bitcast(F32))

    nc.finalize()
    return nc


def build_l2(slot_sizes, psum_bufs=3, wpool_bufs=4, evac_bufs=3):
    """Expert FFN kernel. Per slot s (capacity C): y^T = wgt * (relu(xg^T W1 + b1) W2 + b2)."""
    nc = bacc.Bacc()
    ins = {}
    outs = {}
    for s, C in enumerate(slot_sizes):
        ins[f"xg{s}"] = nc.dram_tensor(f"xg{s}", [128, DC, C], BF16, kind="ExternalInput").ap()
        ins[f"w1_{s}"] = nc.dram_tensor(f"w1_{s}", [FC, 128, DC, 128], BF16, kind="ExternalInput").ap()
        ins[f"w2_{s}"] = nc.dram_tensor(f"w2_{s}", [DC, 128, FC, 128], BF16, kind="ExternalInput").ap()
        ins[f"b1_{s}"] = nc.dram_tensor(f"b1_{s}", [128, FC], F32, kind="ExternalInput").ap()
        ins[f"b2_{s}"] = nc.dram_tensor(f"b2_{s}", [128, DC], F32, kind="ExternalInput").ap()
        ins[f"wg{s}"] = nc.dram_tensor(f"wg{s}", [128, C], F32, kind="ExternalInput").ap()
        outs[f"y{s}"] = nc.dram_tensor(f"y{s}", [DC, 128, C], F32, kind="ExternalOutput").ap()

    with tile.TileContext(nc) as tc:
        with (
            tc.tile_pool(name="act", bufs=1) as act,
            tc.tile_pool(name="wpool", bufs=wpool_bufs) as wpool,
            tc.tile_pool(name="evac", bufs=evac_bufs) as evac,
            tc.tile_pool(name="ps1", bufs=psum_bufs, space="PSUM") as ps1,
            tc.tile_pool(name="ps2", bufs=psum_bufs, space="PSUM") as ps2,
        ):
            for s, C in enumerate(slot_sizes):
                tiles = _token_tiles(C)
                xg_t = act.tile([128, DC, C], BF16, tag="xg")
                nc.sync.dma_start(xg_t, ins[f"xg{s}"])
                wg_t = act.tile([128, C], F32, tag="wg")
                nc.sync.dma_start(wg_t, ins[f"wg{s}"])
                b1_t = act.tile([128, FC], F32, tag="b1")
                nc.sync.dma_start(b1_t, ins[f"b1_{s}"])
                b2_t = act.tile([128, DC], F32, tag="b2")
                nc.sync.dma_start(b2_t, ins[f"b2_{s}"])
                hT = act.tile([128, FC, C], BF16, tag="hT")

                for f in range(FC):
                    w1t = wpool.tile([128, DC, 128], BF16, tag="w1")
                    nc.sync.dma_start(w1t, ins[f"w1_{s}"][f])
                    for (t0, tn) in tiles:
                        ph_full = ps1.tile([128, 512], F32, tag="ph")
                        ph = ph_full[:, :tn]
                        for k in range(DC):
                            nc.tensor.matmul(ph, w1t[:, k, :],
                                             xg_t[:, k, t0 : t0 + tn],
                                             start=(k == 0), stop=(k == DC - 1))
                        nc.scalar.activation(hT[:, f, t0 : t0 + tn], ph,
                                             mybir.ActivationFunctionType.Relu,
                                             bias=b1_t[:, f : f + 1])

                for d in range(DC):
                    w2t = wpool.tile([128, FC, 128], BF16, tag="w2")
                    nc.sync.dma_start(w2t, ins[f"w2_{s}"][d])
                    for (t0, tn) in tiles:
                        py_full = ps2.tile([128, 512], F32, tag="py")
                        py = py_full[:, :tn]
                        for k in range(FC):
                            nc.tensor.matmul(py, w2t[:, k, :],
                                             hT[:, k, t0 : t0 + tn],
                                             start=(k == 0), stop=(k == FC - 1))
                        y32_full = evac.tile([128, 512], F32, tag="y")
                        y32 = y32_full[:, :tn]
                        # y = (py + b2[d]) * wgt
                        nc.vector.scalar_tensor_tensor(y32, py, b2_t[:, d : d + 1],
                                                       wg_t[:, t0 : t0 + tn],
                                                       op0=mybir.AluOpType.add,
                                                       op1=mybir.AluOpType.mult)
                        nc.sync.dma_start(outs[f"y{s}"][d][:, t0 : t0 + tn], y32)

    nc.finalize()
    return nc


def _pack(counts, slot_sizes):
    """Exact splittable bin-packing via DP. Each core has one slot per entry
    of slot_sizes; a slot holds one (expert, n_tokens<=cap) piece. Returns
    {(core, slot_idx): (expert, n_tokens)} or None if infeasible."""
    ns = len(slot_sizes)
    counts = [int(c) for c in counts]
    # options per expert: piece-count vector (per slot size) covering count_e
    def options(cnt):
        opts = []
        maxn = [min(NCORES, -(-cnt // sz)) if cnt else 0 for sz in slot_sizes]
        def rec(i, vec, cap):
            if i == ns:
                if cap >= cnt:
                    opts.append(tuple(vec))
                return
            for n in range(maxn[i] + 1):
                vec.append(n)
                rec(i + 1, vec, cap + n * slot_sizes[i])
                vec.pop()
        rec(0, [], 0)
        # prune dominated options
        opts = [o for o in opts
                if not any(p != o and all(pi <= oi for pi, oi in zip(p, o)) for p in opts)]
        return opts

    state0 = (0,) * ns
    dp = {state0: []}  # state: used slots per size -> list of per-expert vectors
    for e in range(E):
        opts = options(counts[e])
        ndp = {}
        for st, hist in dp.items():
            for o in opts:
                nst = tuple(s + x for s, x in zip(st, o))
                if all(v <= NCORES for v in nst) and nst not in ndp:
                    ndp[nst] = hist + [o]
        dp = ndp
        if not dp:
            return None
    hist = next(iter(dp.values()))
    # build pieces: expert e gets hist[e][si] pieces of slot size si
    slot_fill = {si: [] for si in range(ns)}  # list of (expert, n)
    for e in range(E):
        rem = counts[e]
        pieces = []
        for si in range(ns):
            for _ in range(hist[e][si]):
                pieces.append(si)
        # fill largest pieces first
        pieces.sort(key=lambda si: -slot_sizes[si])
        for si in pieces:
            take = min(slot_sizes[si], rem)
            slot_fill[si].append((e, take))
            rem -= take
        assert rem == 0
    assign = {}
    for si in range(ns):
        fills = slot_fill[si]
        fills += [(0, 0)] * (NCORES - len(fills))
        for c in range(NCORES):
            assign[(c, si)] = fills[c]
    return assign


LAST_KERNELS = {}


def kernel(x, ln_gamma, ln_beta, rW1, rb1, rW2, rb2, We1, be1, We2, be2):
    x = np.asarray(x, np.float32)
    xf = np.ascontiguousarray(x.reshape(NTOK, D))

    # ---------------- Launch 1: router ----------------
    nc1 = build_l1()
    LAST_KERNELS["router"] = nc1
    # fold LayerNorm affine into the router weights (exact):
    #   (z*gamma + beta) @ rW1 + rb1 == z @ (gamma[:,None]*rW1) + (beta@rW1 + rb1)
    gamma = np.asarray(ln_gamma, np.float32)
    beta = np.asarray(ln_beta, np.float32)
    rW1_eff = (np.asarray(rW1, np.float32) * gamma[:, None]).astype(np.float32)
    rb1_eff = (np.asarray(rb1, np.float64)
               + beta.astype(np.float64) @ np.asarray(rW1, np.float64)).astype(np.float32)
    # [m-chunk][p=k%128][k-chunk][col] layout, values pre-rounded to fp32r
    rw1_a = np.ascontiguousarray(
        _rne_f32r(rW1_eff).reshape(DC, 128, DC, 128).transpose(2, 1, 0, 3))
    rw2_pad = np.zeros((D, 128), np.float32)
    rw2_pad[:, :E] = _rne_f32r(np.asarray(rW2, np.float32))
    rw2_a = np.ascontiguousarray(rw2_pad.reshape(DC, 128, 128).transpose(1, 0, 2))
    rb1_a = np.ascontiguousarray(rb1_eff.reshape(DC, 128).T)
    rb2_a = np.asarray(rb2, np.float32).reshape(E, 1)

    common1 = dict(rw1=rw1_a, rw2=rw2_a, rb1=rb1_a, rb2=rb2_a,
                   ident=np.eye(128, dtype=np.float32))
    in_maps1 = [dict(xs=xf[c * TPC:(c + 1) * TPC], **common1) for c in range(NCORES)]
    res1 = run_bass_kernel_spmd(nc1, in_maps1, list(range(NCORES)))
    logits = np.concatenate([res1.results[c]["logits"].T for c in range(NCORES)], axis=0)

    # ---------------- Host routing (O(NTOK*E) control plane) ----------------
    lg = logits.astype(np.float32)
    m = lg.max(axis=1, keepdims=True)
    p = np.exp(lg - m, dtype=np.float32)
    probs = p / p.sum(axis=1, keepdims=True, dtype=np.float32)

    # exact re-route of near-tie tokens (fp32r logits carry ~2e-4 error; any
    # token whose #2/#3 prob gap is below GAP_TH could be mis-routed)
    GAP_TH = 5e-3
    psort = np.sort(probs, axis=1)[:, ::-1]
    risky = np.where(psort[:, 1] - psort[:, 2] < GAP_TH)[0]
    if len(risky):
        xr = xf[risky].astype(np.float64)
        mu = xr.mean(axis=1, keepdims=True)
        var = ((xr - mu) ** 2).mean(axis=1, keepdims=True)
        zn = (xr - mu) / np.sqrt(var + EPS_LN)
        h = np.maximum(zn @ np.asarray(rW1_eff, np.float64) + rb1_eff.astype(np.float64), 0.0)
        lge = h @ np.asarray(rW2, np.float64) + np.asarray(rb2, np.float64).reshape(-1)
        pe_ = np.exp(lge - lge.max(axis=1, keepdims=True))
        probs[risky] = (pe_ / pe_.sum(axis=1, keepdims=True)).astype(np.float32)

    top2 = np.argsort(-probs, axis=1, kind="stable")[:, :TOPK]
    tkp = np.take_along_axis(probs, top2, axis=1)
    wnorm = tkp / tkp.sum(axis=1, keepdims=True, dtype=np.float32)
    p_mean = probs.mean(axis=0, dtype=np.float32)
    aux_loss = np.sum(p_mean * np.log(p_mean * E + EPS_AUX, dtype=np.float32),
                      dtype=np.float32)

    counts = np.bincount(top2.ravel(), minlength=E)

    # ---------------- Pack slots & Launch 2: experts ----------------
    assign = None
    for preset in _slot_candidates():
        assign = _pack(counts, preset)
        if assign is not None:
            slot_sizes = preset
            break
    assert assign is not None, f"no slot preset fits counts {counts}"

    # token lists per expert in order
    tok_by_e = [np.where((top2 == e).any(axis=1))[0] for e in range(E)]
    w_by_e = []
    for e in range(E):
        t = tok_by_e[e]
        sel = np.where(top2[t] == e)  # (row, which-of-2)
        w = np.zeros(len(t), np.float32)
        w[sel[0]] = wnorm[t[sel[0]], sel[1]]
        w_by_e.append(w)

    nc2 = build_l2(slot_sizes)
    LAST_KERNELS["experts"] = nc2

    # per-expert prepared weights (bf16, SBUF layouts), computed lazily
    wcache = {}
    def expert_arrays(e):
        if e not in wcache:
            w1 = np.ascontiguousarray(
                _bf16(We1[e]).reshape(DC, 128, FC, 128).transpose(2, 1, 0, 3))
            w2 = np.ascontiguousarray(
                _bf16(We2[e]).reshape(FC, 128, DC, 128).transpose(2, 1, 0, 3))
            b1 = np.ascontiguousarray(np.asarray(be1[e], np.float32).reshape(FC, 128).T)
            b2 = np.ascontiguousarray(np.asarray(be2[e], np.float32).reshape(DC, 128).T)
            wcache[e] = (w1, w2, b1, b2)
        return wcache[e]

    # consume per-expert token lists sequentially across slots
    offsets = {e: 0 for e in range(E)}
    slot_tokens = {}
    in_maps2 = []
    for c in range(NCORES):
        im = {}
        for si, Csz in enumerate(slot_sizes):
            e, n = assign[(c, si)]
            off = offsets[e]
            toks = tok_by_e[e][off:off + n]
            wg = w_by_e[e][off:off + n]
            offsets[e] = off + n
            slot_tokens[(c, si)] = toks
            xg = np.zeros((Csz, D), np.float32)
            xg[:n] = xf[toks]
            xgb = _bf16(xg)  # [C, D]
            im[f"xg{si}"] = np.ascontiguousarray(
                xgb.T.reshape(DC, 128, Csz).transpose(1, 0, 2))
            w1, w2, b1, b2 = expert_arrays(e)
            im[f"w1_{si}"] = w1
            im[f"w2_{si}"] = w2
            im[f"b1_{si}"] = b1
            im[f"b2_{si}"] = b2
            wgf = np.zeros(Csz, np.float32)
            wgf[:n] = wg
            im[f"wg{si}"] = np.ascontiguousarray(np.broadcast_to(wgf, (128, Csz)))
        in_maps2.append(im)

    res2 = run_bass_kernel_spmd(nc2, in_maps2, list(range(NCORES)))

    # ---------------- Host combine ----------------
    out = np.zeros((NTOK, D), np.float32)
    for c in range(NCORES):
        for si, Csz in enumerate(slot_sizes):
            toks = slot_tokens[(c, si)]
            n = len(toks)
            if n == 0:
                continue
            y = res2.results[c][f"y{si}"]  # [DC, 128, C]
            yt = y.transpose(2, 0, 1).reshape(Csz, D)  # [C, D]
            out[toks] += yt[:n]

    return out.reshape(B, S, D), np.float32(aux_loss)


# revision 45
# speedup vs baseline: 1.0097x; 1.0097x over previous
"""MoE (B=2,S=2048,D=2048,E=8,K=2,F=4096) on 8 Trainium2 NeuronCores.

Strategy:
  Launch 1 (router, data-parallel over tokens): each core LayerNorms its 512
  tokens and computes router logits with bf16 hi/lo 3-pass matmuls (fp32-grade
  accuracy at bf16 PE rate). Softmax / top-2 / gating weights / aux loss are
  tiny O(tokens*E) ops done on host from the logits.
  Host (the "all-to-all"): tokens are grouped by expert, split into slots, and
  bin-packed onto cores to balance the highly non-uniform expert loads.
  Launch 2 (experts, expert-parallel): each core runs the D->F->D FFN (bf16)
  for its assigned (expert, token-chunk) slots and scales by gating weight.
  Host: scatter-add of the two expert contributions per token.
"""
import numpy as np
import ml_dtypes
import concourse.bacc as bacc
import concourse.tile as tile
from concourse import mybir
from concourse.bass_utils import run_bass_kernel_spmd
import concourse.bass as bass

F32 = mybir.dt.float32
BF16 = mybir.dt.bfloat16

B, S, D = 2, 2048, 2048
E, TOPK, F = 8, 2, 4096
EPS_LN = 1e-5
EPS_AUX = 1e-9
NCORES = 8
NTOK = B * S            # 4096
TPC = NTOK // NCORES    # 512 tokens per core in launch 1
DC = D // 128           # 16 chunks of the model dim
FC = F // 128           # 32 chunks of the FFN dim

def _slot_candidates():
    """Slot-size configs (per core, same on all cores), ordered by total
    capacity then by slot count (fewer slots => less weight DMA)."""
    def up16(v):
        return -(-v // 16) * 16

    cands = []
    for total in range(1024, 4096 + 512, 16):
        if total % 64 == 0:
            cands.append((total, 1, (total,)))
        for a in range(up16((total + 1) // 2), total - 192 + 1, 16):
            b = total - a
            if b >= 192:
                cands.append((total, 2, (a, b)))
        if total <= 1600 and total % 32 == 0:
            for a in range(up16((total + 2) // 3), total - 2 * 192 + 1, 32):
                for b in range(up16((total - a + 1) // 2), min(a, total - a - 192) + 1, 32):
                    c = total - a - b
                    if c >= 192:
                        cands.append((total, 3, (a, b, c)))
    cands.sort(key=lambda t: (t[0], t[1]))
    # known-good low-capacity configs first (checked for feasibility at runtime)
    return [(560, 256, 224), (448, 368, 240)] + [c[2] for c in cands]


def _bf16(a):
    return np.asarray(a, np.float32).astype(ml_dtypes.bfloat16)


def _hilo(a):
    hi = _bf16(a)
    lo = (np.asarray(a, np.float32) - hi.astype(np.float32)).astype(ml_dtypes.bfloat16)
    return hi, lo


def _rne_f32r(v):
    """Round fp32 array to fp32r (round-to-nearest-even, 11-bit mantissa)."""
    u = np.ascontiguousarray(v, dtype=np.float32).view(np.uint32).astype(np.uint64)
    shift = np.uint64(12)
    one = np.uint64(1)
    bias = (one << np.uint64(11)) - one + ((u >> shift) & one)
    u2 = ((u + bias) >> shift) << shift
    return u2.astype(np.uint32).view(np.float32)


def _token_tiles(c):
    """Split a slot of c tokens into near-equal PE-friendly tiles (<=512)."""
    nt = -(-c // 512)
    base = c // nt
    base -= base % 16
    sizes = [base] * nt
    sizes[-1] = c - base * (nt - 1)
    tiles = []
    off = 0
    for s in sizes:
        tiles.append((off, s))
        off += s
    return tiles


F32R = mybir.dt.float32r


def build_l1(debug=False, work_bufs=1, small_bufs=2, ps_bufs=2):
    """Router kernel: x_shard [TPC, D] -> logits^T [E, TPC] per core.

    ln_gamma is folded into rW1 and ln_beta into rb1 on the host, so the
    device LayerNorm is a plain z-score. Matmuls run in fp32r (full PE rate,
    ~11-bit mantissa); near-tie tokens are re-routed exactly on the host.
    """
    nc = bacc.Bacc()
    xs = nc.dram_tensor("xs", [TPC, D], F32, kind="ExternalInput").ap()
    # router weights (fp32 values pre-rounded to fp32r on host), prearranged:
    # rw1 [m-chunk 16][p 128][k-chunk 16][col 128]; rw2 [p 128][k-chunk 16][e 8]
    rw1 = nc.dram_tensor("rw1", [DC, 128, DC, 128], F32R, kind="ExternalInput").ap()
    # rW2 zero-padded from E=8 to 128 output columns (fp32r needs M=128)
    rw2 = nc.dram_tensor("rw2", [128, DC, 128], F32R, kind="ExternalInput").ap()
    rb1 = nc.dram_tensor("rb1", [128, DC], F32, kind="ExternalInput").ap()
    rb2 = nc.dram_tensor("rb2", [E, 1], F32, kind="ExternalInput").ap()
    ident = nc.dram_tensor("ident", [128, 128], F32, kind="ExternalInput").ap()
    logits = nc.dram_tensor("logits", [E, TPC], F32, kind="ExternalOutput").ap()
    if debug:
        xnT_o = nc.dram_tensor("xnT_o", [128, DC, TPC], F32, kind="ExternalOutput").ap()
        hr_o = nc.dram_tensor("hr_o", [128, DC, TPC], F32, kind="ExternalOutput").ap()

    TB = TPC // 128  # token blocks of 128

    with tile.TileContext(nc) as tc:
        with (
            tc.tile_pool(name="main", bufs=1) as main,
            tc.tile_pool(name="work", bufs=work_bufs) as work,
            tc.tile_pool(name="small", bufs=small_bufs) as small,
            tc.tile_pool(name="wpool", bufs=2) as wpool,
            tc.tile_pool(name="ps", bufs=ps_bufs, space="PSUM") as ps,
        ):
            # transposed normalized activations (fp32r): [p=d%128, dc, token]
            xnT = main.tile([128, DC, TPC], F32R, tag="xnT")
            id_t = main.tile([128, 128], F32, tag="id_t")
            nc.sync.dma_start(id_t, ident)

            xs_r = xs.rearrange("(tb p) d -> p tb d", p=128)
            for tb in range(TB):
                xtb = small.tile([128, D], F32, tag="xtb")
                nc.sync.dma_start(xtb, xs_r[:, tb, :])
                ssum = small.tile([128, 1], F32, tag="ssum")
                nc.vector.tensor_reduce(ssum, xtb, axis=mybir.AxisListType.X,
                                        op=mybir.AluOpType.add)
                negmu = small.tile([128, 1], F32, tag="negmu")
                nc.vector.tensor_scalar(negmu, ssum, -1.0 / D, None,
                                        op0=mybir.AluOpType.mult)
                xc = work.tile([128, D], F32, tag="xc")
                nc.vector.tensor_scalar(xc, xtb, negmu, None,
                                        op0=mybir.AluOpType.add)
                # var*D via ACT: square with accumulate (frees the DVE)
                sq = work.tile([128, D], F32, tag="sq")
                vsum = small.tile([128, 1], F32, tag="vsum")
                nc.scalar.activation(sq, xc, mybir.ActivationFunctionType.Square,
                                     accum_out=vsum)
                # std = sqrt(vsum/D + eps)
                varep = small.tile([128, 1], F32, tag="varep")
                nc.vector.tensor_scalar(varep, vsum, 1.0 / D, EPS_LN,
                                        op0=mybir.AluOpType.mult,
                                        op1=mybir.AluOpType.add)
                std = small.tile([128, 1], F32, tag="std")
                nc.scalar.sqrt(std, varep)
                rstd = small.tile([128, 1], F32, tag="rstd")
                nc.vector.reciprocal(rstd, std)
                # xn = xc * rstd  (gamma folded into rW1, beta into rb1)
                xn = work.tile([128, D], F32, tag="xn")
                nc.vector.tensor_scalar(xn, xc, rstd, None,
                                        op0=mybir.AluOpType.mult)
                # PE-transpose each [128,128] block; DVE evac rounds to fp32r
                for dc in range(DC):
                    pt = ps.tile([128, 128], F32, tag="pt")
                    nc.tensor.transpose(pt, xn[:, dc * 128:(dc + 1) * 128], id_t)
                    nc.vector.tensor_copy(xnT[:, dc, tb * 128:(tb + 1) * 128], pt)

            # --- mm1: h^T[m] = relu(sum_k rW1[k,m]^T xn^T[k]), fp32r ---
            rb1_t = main.tile([128, DC], F32, tag="rb1")
            nc.sync.dma_start(rb1_t, rb1)
            hr = main.tile([128, DC, TPC], F32R, tag="hr")
            for m in range(DC):
                w1_t = wpool.tile([128, DC, 128], F32R, tag="w1")
                nc.sync.dma_start(w1_t, rw1[m])
                ph = ps.tile([128, TPC], F32, tag="ph")
                for k in range(DC):
                    nc.tensor.matmul(ph, w1_t[:, k, :], xnT[:, k, :],
                                     start=(k == 0), stop=(k == DC - 1))
                # h = max(ph + rb1[m], 0) rounded to fp32r, one DVE op
                nc.vector.tensor_scalar(hr[:, m, :], ph, rb1_t[:, m : m + 1], 0.0,
                                        op0=mybir.AluOpType.add,
                                        op1=mybir.AluOpType.max)

            # --- mm2 fp32r with rW2 zero-padded to M=128 ---
            rw2_t = main.tile([128, DC, 128], F32R, tag="rw2")
            rb2_t = main.tile([E, 1], F32, tag="rb2")
            nc.sync.dma_start(rw2_t, rw2)
            nc.sync.dma_start(rb2_t, rb2)
            pl = ps.tile([128, TPC], F32, tag="pl")
            for k in range(DC):
                nc.tensor.matmul(pl, rw2_t[:, k, :], hr[:, k, :],
                                 start=(k == 0), stop=(k == DC - 1))
            lsb = small.tile([E, TPC], F32, tag="lsb")
            nc.vector.tensor_scalar(lsb, pl[0:E, :], rb2_t[:, 0:1], None,
                                    op0=mybir.AluOpType.add)
            nc.sync.dma_start(logits, lsb)
            if debug:
                nc.sync.dma_start(xnT_o, xnT.bitcast(F32))
                nc.sync.dma_start(hr_o, hr.bitcast(F32))

    nc.finalize()
    return nc


def build_l2(slot_sizes, psum_bufs=3, wpool_bufs=4, evac_bufs=3):
    """Expert FFN kernel. Per slot s (capacity C): y^T = wgt * (relu(xg^T W1 + b1) W2 + b2)."""
    nc = bacc.Bacc()
    ins = {}
    outs = {}
    for s, C in enumerate(slot_sizes):
        ins[f"xg{s}"] = nc.dram_tensor(f"xg{s}", [128, DC, C], BF16, kind="ExternalInput").ap()
        ins[f"w1_{s}"] = nc.dram_tensor(f"w1_{s}", [FC, 128, DC, 128], BF16, kind="ExternalInput").ap()
        ins[f"w2_{s}"] = nc.dram_tensor(f"w2_{s}", [DC, 128, FC, 128], BF16, kind="ExternalInput").ap()
        ins[f"b1_{s}"] = nc.dram_tensor(f"b1_{s}", [128, FC], F32, kind="ExternalInput").ap()
        ins[f"b2_{s}"] = nc.dram_tensor(f"b2_{s}", [128, DC], F32, kind="ExternalInput").ap()
        ins[f"wg{s}"] = nc.dram_tensor(f"wg{s}", [128, C], F32, kind="ExternalInput").ap()
        outs[f"y{s}"] = nc.dram_tensor(f"y{s}", [DC, 128, C], F32, kind="ExternalOutput").ap()

    with tile.TileContext(nc) as tc:
        with (
            tc.tile_pool(name="act", bufs=1) as act,
            tc.tile_pool(name="wpool", bufs=wpool_bufs) as wpool,
            tc.tile_pool(name="evac", bufs=evac_bufs) as evac,
            tc.tile_pool(name="ps1", bufs=psum_bufs, space="PSUM") as ps1,
            tc.tile_pool(name="ps2", bufs=psum_bufs, space="PSUM") as ps2,
        ):
            for s, C in enumerate(slot_sizes):
                tiles = _token_tiles(C)
                xg_t = act.tile([128, DC, C], BF16, tag="xg")
                nc.sync.dma_start(xg_t, ins[f"xg{s}"])
                wg_t = act.tile([128, C], F32, tag="wg")
                nc.sync.dma_start(wg_t, ins[f"wg{s}"])
                b1_t = act.tile([128, FC], F32, tag="b1")
                nc.sync.dma_start(b1_t, ins[f"b1_{s}"])
                b2_t = act.tile([128, DC], F32, tag="b2")
                nc.sync.dma_start(b2_t, ins[f"b2_{s}"])
                hT = act.tile([128, FC, C], BF16, tag="hT")

                for f in range(FC):
                    w1t = wpool.tile([128, DC, 128], BF16, tag="w1")
                    nc.sync.dma_start(w1t, ins[f"w1_{s}"][f])
                    for (t0, tn) in tiles:
                        ph_full = ps1.tile([128, 512], F32, tag="ph")
                        ph = ph_full[:, :tn]
                        for k in range(DC):
                            nc.tensor.matmul(ph, w1t[:, k, :],
                                             xg_t[:, k, t0 : t0 + tn],
                                             start=(k == 0), stop=(k == DC - 1))
                        nc.scalar.activation(hT[:, f, t0 : t0 + tn], ph,
                                             mybir.ActivationFunctionType.Relu,
                                             bias=b1_t[:, f : f + 1])

                for d in range(DC):
                    w2t = wpool.tile([128, FC, 128], BF16, tag="w2")
                    nc.sync.dma_start(w2t, ins[f"w2_{s}"][d])
                    for (t0, tn) in tiles:
                        py_full = ps2.tile([128, 512], F32, tag="py")
                        py = py_full[:, :tn]
                        for k in range(FC):
                            nc.tensor.matmul(py, w2t[:, k, :],
                                             hT[:, k, t0 : t0 + tn],
                                             start=(k == 0), stop=(k == FC - 1))
                        y32_full = evac.tile([128, 512], F32, tag="y")
                        y32 = y32_full[:, :tn]
                        # y = (py + b2[d]) * wgt
                        nc.vector.scalar_tensor_tensor(y32, py, b2_t[:, d : d + 1],
                                                       wg_t[:, t0 : t0 + tn],
                                                       op0=mybir.AluOpType.add,
                                                       op1=mybir.AluOpType.mult)
                        nc.sync.dma_start(outs[f"y{s}"][d][:, t0 : t0 + tn], y32)

    nc.finalize()
    return nc


def _pack(counts, slot_sizes):
    """Exact splittable bin-packing via DP. Each core has one slot per entry
    of slot_sizes; a slot holds one (expert, n_tokens<=cap) piece. Returns
    {(core, slot_idx): (expert, n_tokens)} or None if infeasible."""
    ns = len(slot_sizes)
    counts = [int(c) for c in counts]
    # options per expert: piece-count vector (per slot size) covering count_e
    def options(cnt):
        opts = []
        maxn = [min(NCORES, -(-cnt // sz)) if cnt else 0 for sz in slot_sizes]
        def rec(i, vec, cap):
            if i == ns:
                if cap >= cnt:
                    opts.append(tuple(vec))
                return
            for n in range(maxn[i] + 1):
                vec.append(n)
                rec(i + 1, vec, cap + n * slot_sizes[i])
                vec.pop()
        rec(0, [], 0)
        # prune dominated options
        opts = [o for o in opts
                if not any(p != o and all(pi <= oi for pi, oi in zip(p, o)) for p in opts)]
        return opts

    state0 = (0,) * ns
    dp = {state0: []}  # state: used slots per size -> list of per-expert vectors
    for e in range(E):
        opts = options(counts[e])
        ndp = {}
        for st, hist in dp.items():
            for o in opts:
                nst = tuple(s + x for s, x in zip(st, o))
                if all(v <= NCORES for v in nst) and nst not in ndp:
                    ndp[nst] = hist + [o]
        dp = ndp
        if not dp:
            return None
    hist = next(iter(dp.values()))
    # build pieces: expert e gets hist[e][si] pieces of slot size si
    slot_fill = {si: [] for si in range(ns)}  # list of (expert, n)
    for e in range(E):
        rem = counts[e]
        pieces = []
        for si in range(ns):
            for _ in range(hist[e][si]):
                pieces.append(si)
        # fill largest pieces first
        pieces.sort(key=lambda si: -slot_sizes[si])
        for si in pieces:
            take = min(slot_sizes[si], rem)
            slot_fill[si].append((e, take))
            rem -= take
        assert rem == 0
    assign = {}
    for si in range(ns):
        fills = slot_fill[si]
        fills += [(0, 0)] * (NCORES - len(fills))
        for c in range(NCORES):
            assign[(c, si)] = fills[c]
    return assign


LAST_KERNELS = {}


def kernel(x, ln_gamma, ln_beta, rW1, rb1, rW2, rb2, We1, be1, We2, be2):
    x = np.asarray(x, np.float32)
    xf = np.ascontiguousarray(x.reshape(NTOK, D))

    # ---------------- Launch 1: router ----------------
    nc1 = build_l1()
    LAST_KERNELS["router"] = nc1
    # fold LayerNorm affine into the router weights (exact):
    #   (z*gamma + beta) @ rW1 + rb1 == z @ (gamma[:,None]*rW1) + (beta@rW1 + rb1)
    gamma = np.asarray(ln_gamma, np.float32)
    beta = np.asarray(ln_beta, np.float32)
    rW1_eff = (np.asarray(rW1, np.float32) * gamma[:, None]).astype(np.float32)
    rb1_eff = (np.asarray(rb1, np.float64)
               + beta.astype(np.float64) @ np.asarray(rW1, np.float64)).astype(np.float32)
    # [m-chunk][p=k%128][k-chunk][col] layout, values pre-rounded to fp32r
    rw1_a = np.ascontiguousarray(
        _rne_f32r(rW1_eff).reshape(DC, 128, DC, 128).transpose(2, 1, 0, 3))
    rw2_pad = np.zeros((D, 128), np.float32)
    rw2_pad[:, :E] = _rne_f32r(np.asarray(rW2, np.float32))
    rw2_a = np.ascontiguousarray(rw2_pad.reshape(DC, 128, 128).transpose(1, 0, 2))
    rb1_a = np.ascontiguousarray(rb1_eff.reshape(DC, 128).T)
    rb2_a = np.asarray(rb2, np.float32).reshape(E, 1)

    common1 = dict(rw1=rw1_a, rw2=rw2_a, rb1=rb1_a, rb2=rb2_a,
                   ident=np.eye(128, dtype=np.float32))
    in_maps1 = [dict(xs=xf[c * TPC:(c + 1) * TPC], **common1) for c in range(NCORES)]
    res1 = run_bass_kernel_spmd(nc1, in_maps1, list(range(NCORES)))
    logits = np.concatenate([res1.results[c]["logits"].T for c in range(NCORES)], axis=0)

    # ---------------- Host routing (O(NTOK*E) control plane) ----------------
    lg = logits.astype(np.float32)
    m = lg.max(axis=1, keepdims=True)
    p = np.exp(lg - m, dtype=np.float32)
    probs = p / p.sum(axis=1, keepdims=True, dtype=np.float32)

    # exact re-route of near-tie tokens (fp32r logits carry ~2e-4 error; any
    # token whose #2/#3 prob gap is below GAP_TH could be mis-routed)
    GAP_TH = 5e-3
    psort = np.sort(probs, axis=1)[:, ::-1]
    risky = np.where(psort[:, 1] - psort[:, 2] < GAP_TH)[0]
    if len(risky):
        xr = xf[risky].astype(np.float64)
        mu = xr.mean(axis=1, keepdims=True)
        var = ((xr - mu) ** 2).mean(axis=1, keepdims=True)
        zn = (xr - mu) / np.sqrt(var + EPS_LN)
        h = np.maximum(zn @ np.asarray(rW1_eff, np.float64) + rb1_eff.astype(np.float64), 0.0)
        lge = h @ np.asarray(rW2, np.float64) + np.asarray(rb2, np.float64).reshape(-1)
        pe_ = np.exp(lge - lge.max(axis=1, keepdims=True))
        probs[risky] = (pe_ / pe_.sum(axis=1, keepdims=True)).astype(np.float32)

    top2 = np.argsort(-probs, axis=1, kind="stable")[:, :TOPK]
    tkp = np.take_along_axis(probs, top2, axis=1)
    wnorm = tkp / tkp.sum(axis=1, keepdims=True, dtype=np.float32)
    p_mean = probs.mean(axis=0, dtype=np.float32)
    aux_loss = np.sum(p_mean * np.log(p_mean * E + EPS_AUX, dtype=np.float32),
                      dtype=np.float32)

    counts = np.bincount(top2.ravel(), minlength=E)

    # ---------------- Pack slots & Launch 2: experts ----------------
    assign = None
    for preset in _slot_candidates():
        assign = _pack(counts, preset)
        if assign is not None:
            slot_sizes = preset
            break
    assert assign is not None, f"no slot preset fits counts {counts}"

    # token lists per expert in order
    tok_by_e = [np.where((top2 == e).any(axis=1))[0] for e in range(E)]
    w_by_e = []
    for e in range(E):
        t = tok_by_e[e]
        sel = np.where(top2[t] == e)  # (row, which-of-2)
        w = np.zeros(len(t), np.float32)
        w[sel[0]] = wnorm[t[sel[0]], sel[1]]
        w_by_e.append(w)

    nc2 = build_l2(slot_sizes)
    LAST_KERNELS["experts"] = nc2

    # per-expert prepared weights (bf16, SBUF layouts), computed lazily
    wcache = {}
    def expert_arrays(e):
        if e not in wcache:
            w1 = np.ascontiguousarray(
                _bf16(We1[e]).reshape(DC, 128, FC, 128).transpose(2, 1, 0, 3))
            w2 = np.ascontiguousarray(
                _bf16(We2[e]).reshape(FC, 128, DC, 128).transpose(2, 1, 0, 3))
            b1 = np.ascontiguousarray(np.asarray(be1[e], np.float32).reshape(FC, 128).T)
            b2 = np.ascontiguousarray(np.asarray(be2[e], np.float32).reshape(DC, 128).T)
            wcache[e] = (w1, w2, b1, b2)
        return wcache[e]

    # consume per-expert token lists sequentially across slots
    offsets = {e: 0 for e in range(E)}
    slot_tokens = {}
    in_maps2 = []
    for c in range(NCORES):
        im = {}
        for si, Csz in enumerate(slot_sizes):
            e, n = assign[(c, si)]
            off = offsets[e]
            toks = tok_by_e[e][off:off + n]
            wg = w_by_e[e][off:off + n]
            offsets[e] = off + n
            slot_tokens[(c, si)] = toks
            xg = np.zeros((Csz, D), np.float32)
            xg[:n] = xf[toks]
            xgb = _bf16(xg)  # [C, D]
            im[f"xg{si}"] = np.ascontiguousarray(
                xgb.T.reshape(DC, 128, Csz).transpose(1, 0, 2))
            w1, w2, b1, b2 = expert_arrays(e)
            im[f"w1_{si}"] = w1
            im[f"w2_{si}"] = w2
            im[f"b1_{si}"] = b1
            im[f"b2_{si}"] = b2
            wgf = np.zeros(Csz, np.float32)
            wgf[:n] = wg
            im[f"wg{si}"] = np.ascontiguousarray(np.broadcast_to(wgf, (128, Csz)))
        in_maps2.append(im)

    res2 = run_bass_kernel_spmd(nc2, in_maps2, list(range(NCORES)))

    # ---------------- Host combine ----------------
    out = np.zeros((NTOK, D), np.float32)
    for c in range(NCORES):
        for si, Csz in enumerate(slot_sizes):
            toks = slot_tokens[(c, si)]
            n = len(toks)
            if n == 0:
                continue
            y = res2.results[c][f"y{si}"]  # [DC, 128, C]
            yt = y.transpose(2, 0, 1).reshape(Csz, D)  # [C, D]
            out[toks] += yt[:n]

    return out.reshape(B, S, D), np.float32(aux_loss)


# revision 47
# speedup vs baseline: 1.0161x; 1.0063x over previous
"""MoE (B=2,S=2048,D=2048,E=8,K=2,F=4096) on 8 Trainium2 NeuronCores.

Strategy:
  Launch 1 (router, data-parallel over tokens): each core LayerNorms its 512
  tokens and computes router logits with bf16 hi/lo 3-pass matmuls (fp32-grade
  accuracy at bf16 PE rate). Softmax / top-2 / gating weights / aux loss are
  tiny O(tokens*E) ops done on host from the logits.
  Host (the "all-to-all"): tokens are grouped by expert, split into slots, and
  bin-packed onto cores to balance the highly non-uniform expert loads.
  Launch 2 (experts, expert-parallel): each core runs the D->F->D FFN (bf16)
  for its assigned (expert, token-chunk) slots and scales by gating weight.
  Host: scatter-add of the two expert contributions per token.
"""
import numpy as np
import ml_dtypes
import concourse.bacc as bacc
import concourse.tile as tile
from concourse import mybir
from concourse.bass_utils import run_bass_kernel_spmd
import concourse.bass as bass

F32 = mybir.dt.float32
BF16 = mybir.dt.bfloat16

B, S, D = 2, 2048, 2048
E, TOPK, F = 8, 2, 4096
EPS_LN = 1e-5
EPS_AUX = 1e-9
NCORES = 8
NTOK = B * S            # 4096
TPC = NTOK // NCORES    # 512 tokens per core in launch 1
DC = D // 128           # 16 chunks of the model dim
FC = F // 128           # 32 chunks of the FFN dim

def _slot_candidates():
    """Slot-size configs (per core, same on all cores), ordered by total
    capacity then by slot count (fewer slots => less weight DMA)."""
    def up16(v):
        return -(-v // 16) * 16

    cands = []
    for total in range(1024, 4096 + 512, 16):
        if total % 64 == 0:
            cands.append((total, 1, (total,)))
        for a in range(up16((total + 1) // 2), total - 192 + 1, 16):
            b = total - a
            if b >= 192:
                cands.append((total, 2, (a, b)))
        if total <= 1600 and total % 32 == 0:
            for a in range(up16((total + 2) // 3), total - 2 * 192 + 1, 32):
                for b in range(up16((total - a + 1) // 2), min(a, total - a - 192) + 1, 32):
                    c = total - a - b
                    if c >= 192:
                        cands.append((total, 3, (a, b, c)))
    cands.sort(key=lambda t: (t[0], t[1]))
    # known-good low-capacity configs first (checked for feasibility at runtime)
    return [(560, 256, 224), (448, 368, 240)] + [c[2] for c in cands]


def _bf16(a):
    return np.asarray(a, np.float32).astype(ml_dtypes.bfloat16)


def _hilo(a):
    hi = _bf16(a)
    lo = (np.asarray(a, np.float32) - hi.astype(np.float32)).astype(ml_dtypes.bfloat16)
    return hi, lo


def _rne_f32r(v):
    """Round fp32 array to fp32r (round-to-nearest-even, 11-bit mantissa)."""
    u = np.ascontiguousarray(v, dtype=np.float32).view(np.uint32).astype(np.uint64)
    shift = np.uint64(12)
    one = np.uint64(1)
    bias = (one << np.uint64(11)) - one + ((u >> shift) & one)
    u2 = ((u + bias) >> shift) << shift
    return u2.astype(np.uint32).view(np.float32)


def _token_tiles(c):
    """Split a slot of c tokens into near-equal PE-friendly tiles (<=512)."""
    nt = -(-c // 512)
    base = c // nt
    base -= base % 16
    sizes = [base] * nt
    sizes[-1] = c - base * (nt - 1)
    tiles = []
    off = 0
    for s in sizes:
        tiles.append((off, s))
        off += s
    return tiles


F32R = mybir.dt.float32r


def build_l1(debug=False, work_bufs=1, small_bufs=2, ps_bufs=2):
    """Router kernel: x_shard [TPC, D] -> logits^T [E, TPC] per core.

    ln_gamma is folded into rW1 and ln_beta into rb1 on the host, so the
    device LayerNorm is a plain z-score. Matmuls run in fp32r (full PE rate,
    ~11-bit mantissa); near-tie tokens are re-routed exactly on the host.
    """
    nc = bacc.Bacc()
    xs = nc.dram_tensor("xs", [TPC, D], F32, kind="ExternalInput").ap()
    # router weights (fp32 values pre-rounded to fp32r on host), prearranged:
    # rw1 [m-chunk 16][p 128][k-chunk 16][col 128]; rw2 [p 128][k-chunk 16][e 8]
    rw1 = nc.dram_tensor("rw1", [DC, 128, DC, 128], F32R, kind="ExternalInput").ap()
    # rW2 zero-padded from E=8 to 128 output columns (fp32r needs M=128)
    rw2 = nc.dram_tensor("rw2", [128, DC, 128], F32R, kind="ExternalInput").ap()
    rb1 = nc.dram_tensor("rb1", [128, DC], F32, kind="ExternalInput").ap()
    rb2 = nc.dram_tensor("rb2", [E, 1], F32, kind="ExternalInput").ap()
    ident = nc.dram_tensor("ident", [128, 128], F32, kind="ExternalInput").ap()
    logits = nc.dram_tensor("logits", [E, TPC], F32, kind="ExternalOutput").ap()
    if debug:
        xnT_o = nc.dram_tensor("xnT_o", [128, DC, TPC], F32, kind="ExternalOutput").ap()
        hr_o = nc.dram_tensor("hr_o", [128, DC, TPC], F32, kind="ExternalOutput").ap()

    TB = TPC // 128  # token blocks of 128

    with tile.TileContext(nc) as tc:
        with (
            tc.tile_pool(name="main", bufs=1) as main,
            tc.tile_pool(name="work", bufs=work_bufs) as work,
            tc.tile_pool(name="small", bufs=small_bufs) as small,
            tc.tile_pool(name="wpool", bufs=2) as wpool,
            tc.tile_pool(name="ps", bufs=ps_bufs, space="PSUM") as ps,
        ):
            # transposed normalized activations (fp32r): [p=d%128, dc, token]
            xnT = main.tile([128, DC, TPC], F32R, tag="xnT")
            id_t = main.tile([128, 128], F32, tag="id_t")
            nc.sync.dma_start(id_t, ident)

            xs_r = xs.rearrange("(tb p) d -> p tb d", p=128)
            for tb in range(TB):
                xtb = small.tile([128, D], F32, tag="xtb")
                nc.sync.dma_start(xtb, xs_r[:, tb, :])
                ssum = small.tile([128, 1], F32, tag="ssum")
                nc.vector.tensor_reduce(ssum, xtb, axis=mybir.AxisListType.X,
                                        op=mybir.AluOpType.add)
                negmu = small.tile([128, 1], F32, tag="negmu")
                nc.vector.tensor_scalar(negmu, ssum, -1.0 / D, None,
                                        op0=mybir.AluOpType.mult)
                xc = work.tile([128, D], F32, tag="xc")
                nc.vector.tensor_scalar(xc, xtb, negmu, None,
                                        op0=mybir.AluOpType.add)
                # var*D via ACT: square with accumulate (frees the DVE)
                sq = work.tile([128, D], F32, tag="sq")
                vsum = small.tile([128, 1], F32, tag="vsum")
                nc.scalar.activation(sq, xc, mybir.ActivationFunctionType.Square,
                                     accum_out=vsum)
                # std = sqrt(vsum/D + eps)
                varep = small.tile([128, 1], F32, tag="varep")
                nc.vector.tensor_scalar(varep, vsum, 1.0 / D, EPS_LN,
                                        op0=mybir.AluOpType.mult,
                                        op1=mybir.AluOpType.add)
                std = small.tile([128, 1], F32, tag="std")
                nc.scalar.sqrt(std, varep)
                rstd = small.tile([128, 1], F32, tag="rstd")
                nc.vector.reciprocal(rstd, std)
                # xn = xc * rstd  (gamma folded into rW1, beta into rb1)
                xn = work.tile([128, D], F32, tag="xn")
                nc.vector.tensor_scalar(xn, xc, rstd, None,
                                        op0=mybir.AluOpType.mult)
                # PE-transpose each [128,128] block; DVE evac rounds to fp32r
                for dc in range(DC):
                    pt = ps.tile([128, 128], F32, tag="pt")
                    nc.tensor.transpose(pt, xn[:, dc * 128:(dc + 1) * 128], id_t)
                    nc.vector.tensor_copy(xnT[:, dc, tb * 128:(tb + 1) * 128], pt)

            # --- mm1: h^T[m] = relu(sum_k rW1[k,m]^T xn^T[k]), fp32r ---
            rb1_t = main.tile([128, DC], F32, tag="rb1")
            nc.sync.dma_start(rb1_t, rb1)
            hr = main.tile([128, DC, TPC], F32R, tag="hr")
            for m in range(DC):
                w1_t = wpool.tile([128, DC, 128], F32R, tag="w1")
                nc.sync.dma_start(w1_t, rw1[m])
                ph = ps.tile([128, TPC], F32, tag="ph")
                for k in range(DC):
                    nc.tensor.matmul(ph, w1_t[:, k, :], xnT[:, k, :],
                                     start=(k == 0), stop=(k == DC - 1))
                # h = max(ph + rb1[m], 0) rounded to fp32r, one DVE op
                nc.vector.tensor_scalar(hr[:, m, :], ph, rb1_t[:, m : m + 1], 0.0,
                                        op0=mybir.AluOpType.add,
                                        op1=mybir.AluOpType.max)

            # --- mm2 fp32r with rW2 zero-padded to M=128 ---
            rw2_t = main.tile([128, DC, 128], F32R, tag="rw2")
            rb2_t = main.tile([E, 1], F32, tag="rb2")
            nc.sync.dma_start(rw2_t, rw2)
            nc.sync.dma_start(rb2_t, rb2)
            pl = ps.tile([128, TPC], F32, tag="pl")
            for k in range(DC):
                nc.tensor.matmul(pl, rw2_t[:, k, :], hr[:, k, :],
                                 start=(k == 0), stop=(k == DC - 1))
            lsb = small.tile([E, TPC], F32, tag="lsb")
            nc.vector.tensor_scalar(lsb, pl[0:E, :], rb2_t[:, 0:1], None,
                                    op0=mybir.AluOpType.add)
            nc.sync.dma_start(logits, lsb)
            if debug:
                nc.sync.dma_start(xnT_o, xnT.bitcast(F32))
                nc.sync.dma_start(hr_o, hr.bitcast(F32))

    nc.finalize()
    return nc


def build_l2(slot_sizes, psum_bufs=3, wpool_bufs=6, evac_bufs=3, act_bufs=2):
    """Expert FFN kernel. Per slot s (capacity C): y^T = wgt * (relu(xg^T W1 + b1) W2 + b2)."""
    nc = bacc.Bacc()
    ins = {}
    outs = {}
    for s, C in enumerate(slot_sizes):
        ins[f"xg{s}"] = nc.dram_tensor(f"xg{s}", [128, DC, C], BF16, kind="ExternalInput").ap()
        ins[f"w1_{s}"] = nc.dram_tensor(f"w1_{s}", [FC, 128, DC, 128], BF16, kind="ExternalInput").ap()
        ins[f"w2_{s}"] = nc.dram_tensor(f"w2_{s}", [DC, 128, FC, 128], BF16, kind="ExternalInput").ap()
        ins[f"b1_{s}"] = nc.dram_tensor(f"b1_{s}", [128, FC], F32, kind="ExternalInput").ap()
        ins[f"b2_{s}"] = nc.dram_tensor(f"b2_{s}", [128, DC], F32, kind="ExternalInput").ap()
        ins[f"wg{s}"] = nc.dram_tensor(f"wg{s}", [128, C], F32, kind="ExternalInput").ap()
        outs[f"y{s}"] = nc.dram_tensor(f"y{s}", [DC, 128, C], F32, kind="ExternalOutput").ap()

    with tile.TileContext(nc) as tc:
        with (
            tc.tile_pool(name="act", bufs=act_bufs) as act,
            tc.tile_pool(name="wpool", bufs=wpool_bufs) as wpool,
            tc.tile_pool(name="evac", bufs=evac_bufs) as evac,
            tc.tile_pool(name="ps1", bufs=psum_bufs, space="PSUM") as ps1,
            tc.tile_pool(name="ps2", bufs=psum_bufs, space="PSUM") as ps2,
        ):
            for s, C in enumerate(slot_sizes):
                tiles = _token_tiles(C)
                xg_t = act.tile([128, DC, C], BF16, tag="xg")
                nc.sync.dma_start(xg_t, ins[f"xg{s}"])
                wg_t = act.tile([128, C], F32, tag="wg")
                nc.sync.dma_start(wg_t, ins[f"wg{s}"])
                b1_t = act.tile([128, FC], F32, tag="b1")
                nc.sync.dma_start(b1_t, ins[f"b1_{s}"])
                b2_t = act.tile([128, DC], F32, tag="b2")
                nc.sync.dma_start(b2_t, ins[f"b2_{s}"])
                hT = act.tile([128, FC, C], BF16, tag="hT")

                for f in range(FC):
                    w1t = wpool.tile([128, DC, 128], BF16, tag="w1")
                    nc.sync.dma_start(w1t, ins[f"w1_{s}"][f])
                    for (t0, tn) in tiles:
                        ph_full = ps1.tile([128, 512], F32, tag="ph")
                        ph = ph_full[:, :tn]
                        for k in range(DC):
                            nc.tensor.matmul(ph, w1t[:, k, :],
                                             xg_t[:, k, t0 : t0 + tn],
                                             start=(k == 0), stop=(k == DC - 1))
                        nc.scalar.activation(hT[:, f, t0 : t0 + tn], ph,
                                             mybir.ActivationFunctionType.Relu,
                                             bias=b1_t[:, f : f + 1])

                for d in range(DC):
                    w2t = wpool.tile([128, FC, 128], BF16, tag="w2")
                    nc.sync.dma_start(w2t, ins[f"w2_{s}"][d])
                    for (t0, tn) in tiles:
                        py_full = ps2.tile([128, 512], F32, tag="py")
                        py = py_full[:, :tn]
                        for k in range(FC):
                            nc.tensor.matmul(py, w2t[:, k, :],
                                             hT[:, k, t0 : t0 + tn],
                                             start=(k == 0), stop=(k == FC - 1))
                        y32_full = evac.tile([128, 512], F32, tag="y")
                        y32 = y32_full[:, :tn]
                        # y = (py + b2[d]) * wgt
                        nc.vector.scalar_tensor_tensor(y32, py, b2_t[:, d : d + 1],
                                                       wg_t[:, t0 : t0 + tn],
                                                       op0=mybir.AluOpType.add,
                                                       op1=mybir.AluOpType.mult)
                        nc.sync.dma_start(outs[f"y{s}"][d][:, t0 : t0 + tn], y32)

    nc.finalize()
    return nc


def _pack(counts, slot_sizes):
    """Exact splittable bin-packing via DP. Each core has one slot per entry
    of slot_sizes; a slot holds one (expert, n_tokens<=cap) piece. Returns
    {(core, slot_idx): (expert, n_tokens)} or None if infeasible."""
    ns = len(slot_sizes)
    counts = [int(c) for c in counts]
    # options per expert: piece-count vector (per slot size) covering count_e
    def options(cnt):
        opts = []
        maxn = [min(NCORES, -(-cnt // sz)) if cnt else 0 for sz in slot_sizes]
        def rec(i, vec, cap):
            if i == ns:
                if cap >= cnt:
                    opts.append(tuple(vec))
                return
            for n in range(maxn[i] + 1):
                vec.append(n)
                rec(i + 1, vec, cap + n * slot_sizes[i])
                vec.pop()
        rec(0, [], 0)
        # prune dominated options
        opts = [o for o in opts
                if not any(p != o and all(pi <= oi for pi, oi in zip(p, o)) for p in opts)]
        return opts

    state0 = (0,) * ns
    dp = {state0: []}  # state: used slots per size -> list of per-expert vectors
    for e in range(E):
        opts = options(counts[e])
        ndp = {}
        for st, hist in dp.items():
            for o in opts:
                nst = tuple(s + x for s, x in zip(st, o))
                if all(v <= NCORES for v in nst) and nst not in ndp:
                    ndp[nst] = hist + [o]
        dp = ndp
        if not dp:
            return None
    hist = next(iter(dp.values()))
    # build pieces: expert e gets hist[e][si] pieces of slot size si
    slot_fill = {si: [] for si in range(ns)}  # list of (expert, n)
    for e in range(E):
        rem = counts[e]
        pieces = []
        for si in range(ns):
            for _ in range(hist[e][si]):
                pieces.append(si)
        # fill largest pieces first
        pieces.sort(key=lambda si: -slot_sizes[si])
        for si in pieces:
            take = min(slot_sizes[si], rem)
            slot_fill[si].append((e, take))
            rem -= take
        assert rem == 0
    assign = {}
    for si in range(ns):
        fills = slot_fill[si]
        fills += [(0, 0)] * (NCORES - len(fills))
        for c in range(NCORES):
            assign[(c, si)] = fills[c]
    return assign


LAST_KERNELS = {}


def kernel(x, ln_gamma, ln_beta, rW1, rb1, rW2, rb2, We1, be1, We2, be2):
    x = np.asarray(x, np.float32)
    xf = np.ascontiguousarray(x.reshape(NTOK, D))

    # ---------------- Launch 1: router ----------------
    nc1 = build_l1()
    LAST_KERNELS["router"] = nc1
    # fold LayerNorm affine into the router weights (exact):
    #   (z*gamma + beta) @ rW1 + rb1 == z @ (gamma[:,None]*rW1) + (beta@rW1 + rb1)
    gamma = np.asarray(ln_gamma, np.float32)
    beta = np.asarray(ln_beta, np.float32)
    rW1_eff = (np.asarray(rW1, np.float32) * gamma[:, None]).astype(np.float32)
    rb1_eff = (np.asarray(rb1, np.float64)
               + beta.astype(np.float64) @ np.asarray(rW1, np.float64)).astype(np.float32)
    # [m-chunk][p=k%128][k-chunk][col] layout, values pre-rounded to fp32r
    rw1_a = np.ascontiguousarray(
        _rne_f32r(rW1_eff).reshape(DC, 128, DC, 128).transpose(2, 1, 0, 3))
    rw2_pad = np.zeros((D, 128), np.float32)
    rw2_pad[:, :E] = _rne_f32r(np.asarray(rW2, np.float32))
    rw2_a = np.ascontiguousarray(rw2_pad.reshape(DC, 128, 128).transpose(1, 0, 2))
    rb1_a = np.ascontiguousarray(rb1_eff.reshape(DC, 128).T)
    rb2_a = np.asarray(rb2, np.float32).reshape(E, 1)

    common1 = dict(rw1=rw1_a, rw2=rw2_a, rb1=rb1_a, rb2=rb2_a,
                   ident=np.eye(128, dtype=np.float32))
    in_maps1 = [dict(xs=xf[c * TPC:(c + 1) * TPC], **common1) for c in range(NCORES)]
    res1 = run_bass_kernel_spmd(nc1, in_maps1, list(range(NCORES)))
    logits = np.concatenate([res1.results[c]["logits"].T for c in range(NCORES)], axis=0)

    # ---------------- Host routing (O(NTOK*E) control plane) ----------------
    lg = logits.astype(np.float32)
    m = lg.max(axis=1, keepdims=True)
    p = np.exp(lg - m, dtype=np.float32)
    probs = p / p.sum(axis=1, keepdims=True, dtype=np.float32)

    # exact re-route of near-tie tokens (fp32r logits carry ~2e-4 error; any
    # token whose #2/#3 prob gap is below GAP_TH could be mis-routed)
    GAP_TH = 5e-3
    psort = np.sort(probs, axis=1)[:, ::-1]
    risky = np.where(psort[:, 1] - psort[:, 2] < GAP_TH)[0]
    if len(risky):
        xr = xf[risky].astype(np.float64)
        mu = xr.mean(axis=1, keepdims=True)
        var = ((xr - mu) ** 2).mean(axis=1, keepdims=True)
        zn = (xr - mu) / np.sqrt(var + EPS_LN)
        h = np.maximum(zn @ np.asarray(rW1_eff, np.float64) + rb1_eff.astype(np.float64), 0.0)
        lge = h @ np.asarray(rW2, np.float64) + np.asarray(rb2, np.float64).reshape(-1)
        pe_ = np.exp(lge - lge.max(axis=1, keepdims=True))
        probs[risky] = (pe_ / pe_.sum(axis=1, keepdims=True)).astype(np.float32)

    top2 = np.argsort(-probs, axis=1, kind="stable")[:, :TOPK]
    tkp = np.take_along_axis(probs, top2, axis=1)
    wnorm = tkp / tkp.sum(axis=1, keepdims=True, dtype=np.float32)
    p_mean = probs.mean(axis=0, dtype=np.float32)
    aux_loss = np.sum(p_mean * np.log(p_mean * E + EPS_AUX, dtype=np.float32),
                      dtype=np.float32)

    counts = np.bincount(top2.ravel(), minlength=E)

    # ---------------- Pack slots & Launch 2: experts ----------------
    assign = None
    for preset in _slot_candidates():
        assign = _pack(counts, preset)
        if assign is not None:
            slot_sizes = preset
            break
    assert assign is not None, f"no slot preset fits counts {counts}"

    # token lists per expert in order
    tok_by_e = [np.where((top2 == e).any(axis=1))[0] for e in range(E)]
    w_by_e = []
    for e in range(E):
        t = tok_by_e[e]
        sel = np.where(top2[t] == e)  # (row, which-of-2)
        w = np.zeros(len(t), np.float32)
        w[sel[0]] = wnorm[t[sel[0]], sel[1]]
        w_by_e.append(w)

    nc2 = build_l2(slot_sizes)
    LAST_KERNELS["experts"] = nc2

    # per-expert prepared weights (bf16, SBUF layouts), computed lazily
    wcache = {}
    def expert_arrays(e):
        if e not in wcache:
            w1 = np.ascontiguousarray(
                _bf16(We1[e]).reshape(DC, 128, FC, 128).transpose(2, 1, 0, 3))
            w2 = np.ascontiguousarray(
                _bf16(We2[e]).reshape(FC, 128, DC, 128).transpose(2, 1, 0, 3))
            b1 = np.ascontiguousarray(np.asarray(be1[e], np.float32).reshape(FC, 128).T)
            b2 = np.ascontiguousarray(np.asarray(be2[e], np.float32).reshape(DC, 128).T)
            wcache[e] = (w1, w2, b1, b2)
        return wcache[e]

    # consume per-expert token lists sequentially across slots
    offsets = {e: 0 for e in range(E)}
    slot_tokens = {}
    in_maps2 = []
    for c in range(NCORES):
        im = {}
        for si, Csz in enumerate(slot_sizes):
            e, n = assign[(c, si)]
            off = offsets[e]
            toks = tok_by_e[e][off:off + n]
            wg = w_by_e[e][off:off + n]
            offsets[e] = off + n
            slot_tokens[(c, si)] = toks
            xg = np.zeros((Csz, D), np.float32)
            xg[:n] = xf[toks]
            xgb = _bf16(xg)  # [C, D]
            im[f"xg{si}"] = np.ascontiguousarray(
                xgb.T.reshape(DC, 128, Csz).transpose(1, 0, 2))
            w1, w2, b1, b2 = expert_arrays(e)
            im[f"w1_{si}"] = w1
            im[f"w2_{si}"] = w2
            im[f"b1_{si}"] = b1
            im[f"b2_{si}"] = b2
            wgf = np.zeros(Csz, np.float32)
            wgf[:n] = wg
            im[f"wg{si}"] = np.ascontiguousarray(np.broadcast_to(wgf, (128, Csz)))
        in_maps2.append(im)

    res2 = run_bass_kernel_spmd(nc2, in_maps2, list(range(NCORES)))

    # ---------------- Host combine ----------------
    out = np.zeros((NTOK, D), np.float32)
    for c in range(NCORES):
        for si, Csz in enumerate(slot_sizes):
            toks = slot_tokens[(c, si)]
            n = len(toks)
            if n == 0:
                continue
            y = res2.results[c][f"y{si}"]  # [DC, 128, C]
            yt = y.transpose(2, 0, 1).reshape(Csz, D)  # [C, D]
            out[toks] += yt[:n]

    return out.reshape(B, S, D), np.float32(aux_loss)
